# revision 31
# baseline (speedup 1.0000x reference)
"""Trainium2 Bass kernel for nn_CAttention (sparse cluster attention).

Contract: kernel(**inputs) takes FULL unsharded numpy inputs and returns the
full output tuple (x_out [8,2048,512] f32, attn_map [8,2048,2048] f32).

Strategy: data-parallel over batch B=8 across the 8 NeuronCores. Per core:
  - attn_map = (x@Wq*SCALE) @ (x@Wk)^T computed dense (required output).
  - The attn @ v_s product collapses: within a cluster every row of the
    masked/normalized attn matrix is identical, so the [N,N]@[N,C] product
    reduces to 16 per-cluster weighted sums of v rows (rank-16), and the
    output projection factors through Pt = St^T @ Wproj (host-precomputed).
  - All permutations (argsort shuffle, the swapaxes/reshape scramble, the
    restore gather) are folded into host-side input prep / output gather.
TensorEngine streams run in bf16 (full rate, halves DMA/SBUF); accumulation
is fp32 in PSUM; softmax/normalization arithmetic is fp32.
"""

import numpy as np

import concourse.bass as bass
import concourse.tile as tile
from concourse import bacc, mybir

N = 2048
C = 512
NCL = 16
EPS = 1e-6
EPSN = EPS / N
SCALE = (C // 8) ** -0.5  # HEAD_DIM = 64

f32 = mybir.dt.float32
bf16 = mybir.dt.bfloat16

KC = C // 128   # 4 contraction chunks over channel dim
IC = N // 128   # 16 row chunks over token dim
JB = N // 512   # 4 free-dim blocks of 512 over tokens


def _build_nc(rep=1):
    nc = bacc.Bacc("TRN2", target_bir_lowering=False, debug=False, num_devices=8)

    # xcat[k] = [xst_k | xsst_k | xt_k] column-concatenated per 128-row chunk;
    # wcat[k] = [wq_k | wk_k | wv_k]; m1sr pre-interleaved [128, IC*NCL];
    # ptc = pt flattened [17, 4*C]; ehc = [epsc | hones] [128, 18]
    xcat = nc.dram_tensor("xcat", [KC, 128, 3 * N], bf16, kind="ExternalInput")
    wcat = nc.dram_tensor("wcat", [KC, 128, 3 * C], bf16, kind="ExternalInput")
    m1sr = nc.dram_tensor("m1sr", [128, IC * NCL], f32, kind="ExternalInput")
    ehc = nc.dram_tensor("ehc", [128, NCL + 2], bf16, kind="ExternalInput")
    ones1 = nc.dram_tensor("ones1", [1, C], bf16, kind="ExternalInput")
    ptc = nc.dram_tensor("ptc", [NCL + 1, 4 * C], bf16, kind="ExternalInput")
    attn = nc.dram_tensor("attn", [N, N], f32, kind="ExternalOutput")
    z = nc.dram_tensor("z", [N, C], f32, kind="ExternalOutput")

    with tile.TileContext(nc) as tc:
        for _r in range(rep):
            with (
                tc.tile_pool(name=f"pers{_r}", bufs=1) as pc,
                tc.tile_pool(name=f"work{_r}", bufs=3) as pw,
                tc.tile_pool(name=f"psum{_r}", bufs=1, space="PSUM") as pp,
            ):
                xc_t, wc_t = [], []
                qT_t, kT_t = [], []
                for k in range(KC):
                    xc_t.append(pc.tile([128, 3 * N], bf16, tag=f"xc{k}",
                                        name=f"xc{k}"))
                    wc_t.append(pc.tile([128, 3 * C], bf16, tag=f"wc{k}",
                                        name=f"wc{k}"))
                    qT_t.append(pc.tile([128, N], bf16, tag=f"qT{k}", name=f"qT{k}"))
                    kT_t.append(pc.tile([128, N], bf16, tag=f"kT{k}", name=f"kT{k}"))
                xst_t = [t[:, 0:N] for t in xc_t]
                xsst_t = [t[:, N:2 * N] for t in xc_t]
                xt_t = [t[:, 2 * N:3 * N] for t in xc_t]
                wv_t = [t[:, 0:C] for t in wc_t]
                wq_t = [t[:, C:2 * C] for t in wc_t]
                wk_t = [t[:, 2 * C:3 * C] for t in wc_t]

                # ---- loads: few large DMAs, first-consumption order ----
                for k in range(KC):
                    nc.scalar.dma_start(wc_t[k][:], wcat[k])
                for k in range(KC):
                    nc.sync.dma_start(xc_t[k][:, 0:512], xcat[k, :, 0:512])
                for k in range(KC):
                    nc.sync.dma_start(xc_t[k][:, N:N + 512], xcat[k, :, N:N + 512])
                for k in range(KC):
                    nc.sync.dma_start(xc_t[k][:, 2 * N:2 * N + 512],
                                      xcat[k, :, 2 * N:2 * N + 512])
                for k in range(KC):
                    nc.sync.dma_start(xc_t[k][:, 512:N], xcat[k, :, 512:N])
                for k in range(KC):
                    nc.sync.dma_start(xc_t[k][:, N + 512:2 * N],
                                      xcat[k, :, N + 512:2 * N])
                for k in range(KC):
                    nc.sync.dma_start(xc_t[k][:, 2 * N + 512:3 * N],
                                      xcat[k, :, 2 * N + 512:3 * N])
                m1s_sb = pc.tile([128, IC * NCL], f32, tag="m1s", name="m1s_sb")
                nc.scalar.dma_start(m1s_sb[:], m1sr[:])
                eh_sb = pc.tile([128, NCL + 2], bf16, tag="ehc", name="eh_sb")
                nc.scalar.dma_start(eh_sb[:], ehc[:])
                epsc_sb = eh_sb[:, 0:NCL]
                hones_sb = eh_sb[:, NCL:NCL + 2]
                ptc_sb = pc.tile([NCL + 1, 4 * C], bf16, tag="ptc", name="ptc_sb")
                nc.scalar.dma_start(ptc_sb[:], ptc[:])
                pt_sb = [ptc_sb[:, t4 * C:(t4 + 1) * C] for t4 in range(4)]
                o17 = pc.tile([NCL + 1, C], bf16, tag="o17", name="o17")
                nc.scalar.dma_start(o17[NCL:NCL + 1, :], ones1[:])

                ws_ps = pp.tile([NCL, C], f32, tag="ws", name="ws_ps")
                s_ps = pp.tile([NCL, 2], f32, tag="s", name="s_ps")

                # ---- phase 1 ic-loop with interleaved qT/kT projection ----
                proj_pairs = [(m, j) for m in range(KC) for j in range(JB)]
                for i in range(IC):
                    sl = bass.ts(i, 128)
                    ps_v = pp.tile([128, C], f32, tag="psv", name="ps_v")
                    ps_q = pp.tile([128, C], f32, tag="psq", name="ps_q")
                    ps_k = pp.tile([128, C], f32, tag="psk", name="ps_k")
                    for k in range(KC):
                        nc.tensor.matmul(ps_v[:], xst_t[k][:, sl], wv_t[k],
                                         start=(k == 0), stop=(k == KC - 1))
                    for k in range(KC):
                        nc.tensor.matmul(ps_q[:], xsst_t[k][:, sl], wq_t[k][:],
                                         start=(k == 0), stop=(k == KC - 1))
                    for k in range(KC):
                        nc.tensor.matmul(ps_k[:], xt_t[k][:, sl], wk_t[k][:],
                                         start=(k == 0), stop=(k == KC - 1))
                    # rowdot -> colv; exp -> e  (fp32 throughout)
                    # (tensor_tensor_reduce hangs on HW; use copy+mul+reduce)
                    qdd = pw.tile([128, C], f32, tag="qdd", name="qdd")
                    nc.vector.tensor_copy(qdd[:], ps_q[:])
                    nc.vector.tensor_mul(qdd[:], qdd[:], ps_k[:])
                    colv = pw.tile([128, 1], f32, tag="colv", name="colv")
                    nc.vector.reduce_sum(colv[:], qdd[:], axis=mybir.AxisListType.X)
                    e_sb = pw.tile([128, 1], f32, tag="e", name="e_sb")
                    nc.scalar.activation(e_sb[:], colv[:],
                                         mybir.ActivationFunctionType.Exp)
                    m1e = pw.tile([128, NCL], bf16, tag="m1e", name="m1e")
                    nc.vector.tensor_scalar_mul(
                        m1e[:], m1s_sb[:, bass.ts(i, NCL)], e_sb[:])
                    vs = pw.tile([128, C], bf16, tag="vs", name="vs")
                    nc.scalar.activation(vs[:], ps_v[:],
                                         mybir.ActivationFunctionType.Copy)
                    nc.tensor.matmul(ws_ps[:], m1e[:], vs[:],
                                     start=(i == 0), stop=False,
                                     skip_group_check=True)
                    nc.tensor.matmul(ws_ps[:], epsc_sb[:], vs[:],
                                     start=False, stop=(i == IC - 1),
                                     skip_group_check=True)
                    nc.tensor.matmul(s_ps[:], m1e[:], hones_sb[:],
                                     start=(i == 0), stop=(i == IC - 1),
                                     skip_group_check=True)
                    # interleave one qT/kT projection (m, j) pair per ic
                    m, j = proj_pairs[i]
                    slj = bass.ts(j, 512)
                    psp = pp.tile([128, 512], f32, tag="mm", bufs=3, name="psp")
                    for k in range(KC):
                        nc.tensor.matmul(psp[:], wq_t[k][:, bass.ts(m, 128)],
                                         xt_t[k][:, slj],
                                         start=(k == 0), stop=(k == KC - 1))
                    nc.scalar.activation(qT_t[m][:, slj], psp[:],
                                         mybir.ActivationFunctionType.Copy)
                    psp2 = pp.tile([128, 512], f32, tag="mm", bufs=3, name="psp2")
                    for k in range(KC):
                        nc.tensor.matmul(psp2[:], wk_t[k][:, bass.ts(m, 128)],
                                         xt_t[k][:, slj],
                                         start=(k == 0), stop=(k == KC - 1))
                    nc.vector.tensor_copy(kT_t[m][:, slj], psp2[:])

                # ---- O = (WS + eps-term) / (S + EPS); bias row is 1.0 ----
                s_eps = pw.tile([NCL, 1], f32, tag="seps", name="s_eps")
                nc.vector.tensor_scalar_add(s_eps[:], s_ps[:, 0:1], EPS)
                recip = pw.tile([NCL, 1], f32, tag="recip", name="recip")
                nc.vector.reciprocal(recip[:], s_eps[:])
                nc.vector.tensor_scalar_mul(o17[0:NCL, :], ws_ps[:], recip[:])

                # ---- attn_map = qT^T @ kT, streamed out (merged DMA) ----
                # z units (Z_t = O'^T @ Pt, rank-17) interleaved into the loop
                zv = z[:].rearrange("(p t) m -> t p m", t=4)
                zunits = [(t4, pck) for t4 in range(4) for pck in range(KC)]
                for i in range(IC):
                    sli = bass.ts(i, 128)
                    asb = pw.tile([128, N], f32, tag="asb", bufs=4, name="asb")
                    for j in range(JB):
                        slj = bass.ts(j, 512)
                        psa = pp.tile([128, 512], f32, tag="mm", bufs=3,
                                      name="psa")
                        for m in range(KC):
                            nc.tensor.matmul(psa[:], qT_t[m][:, sli],
                                             kT_t[m][:, slj],
                                             start=(m == 0), stop=(m == KC - 1))
                        if j % 2 == 0:
                            nc.vector.tensor_copy(asb[:, slj], psa[:])
                        else:
                            nc.scalar.activation(
                                asb[:, slj], psa[:],
                                mybir.ActivationFunctionType.Copy)
                    if i >= IC - 2:
                        nc.sync.dma_start(attn[sli, 0:1024], asb[:, 0:1024])
                        nc.scalar.dma_start(attn[sli, 1024:N], asb[:, 1024:N])
                    elif i % 2 == 0:
                        nc.sync.dma_start(attn[sli, :], asb[:])
                    else:
                        nc.scalar.dma_start(attn[sli, :], asb[:])
                    if i < 8:
                        for t4, pck in (zunits[2 * i], zunits[2 * i + 1]):
                            psz = pp.tile([128, C], f32, tag="mm", bufs=3,
                                          name="psz")
                            nc.tensor.matmul(psz[:], o17[:, bass.ts(pck, 128)],
                                             pt_sb[t4], start=True, stop=True)
                            zsb = pw.tile([128, C], f32, tag="zsb", bufs=2,
                                          name="zsb")
                            if pck % 2 == 0:
                                nc.vector.tensor_copy(zsb[:], psz[:])
                            else:
                                nc.scalar.activation(
                                    zsb[:], psz[:],
                                    mybir.ActivationFunctionType.Copy)
                            nc.sync.dma_start(zv[t4, bass.ts(pck, 128), :],
                                              zsb[:])

    nc.compile()
    return nc


def _make_runner(nc, n_cores=8):
    import jax
    from jax.sharding import Mesh, PartitionSpec
    from jax.experimental.shard_map import shard_map
    from concourse import bass2jax
    from concourse.bass2jax import _bass_exec_p, install_neuronx_cc_hook

    install_neuronx_cc_hook()
    partition_name = nc.partition_id_tensor.name if nc.partition_id_tensor else None
    in_names, out_names, out_avals, zero_outs = [], [], [], []
    for alloc in nc.m.functions[0].allocations:
        if not isinstance(alloc, mybir.MemoryLocationSet):
            continue
        name = alloc.memorylocations[0].name
        if alloc.kind == "ExternalInput":
            if name != partition_name:
                in_names.append(name)
        elif alloc.kind == "ExternalOutput":
            out_names.append(name)
            shape = tuple(alloc.tensor_shape)
            dtype = mybir.dt.np(alloc.dtype)
            out_avals.append(jax.core.ShapedArray(shape, dtype))
            zero_outs.append(np.zeros(shape, dtype))
    n_params = len(in_names)
    n_outs = len(out_avals)
    all_in_names = list(in_names) + list(out_names)
    if partition_name is not None:
        all_in_names.append(partition_name)

    def _body(*args):
        operands = list(args)
        if partition_name is not None:
            operands.append(bass2jax.partition_id_tensor())
        outs = _bass_exec_p.bind(
            *operands,
            out_avals=tuple(out_avals),
            in_names=tuple(all_in_names),
            out_names=tuple(out_names),
            lowering_input_output_aliases=(),
            sim_require_finite=True,
            sim_require_nnan=True,
            nc=nc,
        )
        return tuple(outs)

    devices = jax.devices()[:n_cores]
    mesh = Mesh(np.asarray(devices), ("core",))
    in_specs = (PartitionSpec("core"),) * (n_params + n_outs)
    out_specs = (PartitionSpec("core"),) * n_outs
    sharded = jax.jit(
        shard_map(_body, mesh=mesh, in_specs=in_specs, out_specs=out_specs,
                  check_rep=False),
        keep_unused=True,
    )

    def run(in_maps):
        per_core = [[np.asarray(m[name]) for name in in_names] for m in in_maps]
        concat_in = [
            np.concatenate([per_core[cc][i] for cc in range(n_cores)], axis=0)
            for i in range(n_params)
        ]
        concat_zeros = [
            np.zeros((n_cores * zz.shape[0], *zz.shape[1:]), zz.dtype)
            for zz in zero_outs
        ]
        out_arrs = sharded(*concat_in, *concat_zeros)
        import jax as _jax
        _jax.block_until_ready(out_arrs)
        return [
            {name: np.asarray(out_arrs[i]).reshape(n_cores, *out_avals[i].shape)[cc]
             for i, name in enumerate(out_names)}
            for cc in range(n_cores)
        ]

    return run


_STATE = {}


def _get_runner():
    if "run" not in _STATE:
        nc = _build_nc()
        _STATE["nc"] = nc
        _STATE["run"] = _make_runner(nc, 8)
    return _STATE["run"]


def kernel(x_token, x_path, idx_cluster, cluster_num, Wqk, Wv, Wpv, Wproj, bproj):
    import ml_dtypes
    bf = ml_dtypes.bfloat16
    x = np.asarray(x_token, dtype=np.float32)
    idx = np.asarray(idx_cluster)
    B = x.shape[0]
    cn = int(cluster_num)
    Wq = np.ascontiguousarray(np.asarray(Wqk, np.float32)[:, :C] * SCALE)
    Wk = np.ascontiguousarray(np.asarray(Wqk, np.float32)[:, C:])
    Wv_ = np.asarray(Wv, np.float32)
    Wp = np.asarray(Wproj, np.float32)
    bp = np.asarray(bproj, np.float32)
    # wcat[k] = [wq_k | wk_k | wv_k]  (bf16)
    wcat = np.empty((KC, 128, 3 * C), np.float32)
    for k in range(KC):
        r = slice(k * 128, (k + 1) * 128)
        wcat[k, :, 0:C] = Wv_[r]
        wcat[k, :, C:2 * C] = Wq[r]
        wcat[k, :, 2 * C:3 * C] = Wk[r]
    wcat = wcat.astype(bf)
    ehc = np.concatenate([np.full((128, NCL), EPSN, np.float32),
                          np.ones((128, 2), np.float32)], axis=1).astype(bf)
    ones1_a = np.ones((1, C), bf)

    in_maps = []
    rhos = []
    for b in range(B):
        sig = np.argsort(idx[b], kind="stable")
        rho = np.argsort(sig, kind="stable")
        s = idx[b][sig]
        xb = x[b]
        xT = xb.T
        xcat = np.empty((KC, 128, 3 * N), np.float32)
        xsT = xb[sig].T
        xssT = xb[sig[sig]].T
        for k in range(KC):
            r = slice(k * 128, (k + 1) * 128)
            xcat[k, :, 0:N] = xsT[r]
            xcat[k, :, N:2 * N] = xssT[r]
            xcat[k, :, 2 * N:3 * N] = xT[r]
        xcat = xcat.astype(bf)
        m1 = np.zeros((N, NCL), np.float32)
        act = s < cn
        m1[np.nonzero(act)[0], s[act]] = 1.0
        # pre-interleave: m1sr[p, i*NCL + c] = m1[i*128 + p, c]
        m1sr = np.ascontiguousarray(
            m1.reshape(IC, 128, NCL).transpose(1, 0, 2).reshape(128, IC * NCL))
        ptb = np.zeros((NCL + 1, 4 * C), np.float32)
        for t4 in range(4):
            st = s[t4 * 512:(t4 + 1) * 512]
            blk = np.zeros((NCL + 1, C), np.float32)
            np.add.at(blk, st, Wp)
            blk[NCL] = bp
            ptb[:, t4 * C:(t4 + 1) * C] = blk
        in_maps.append({
            "xcat": xcat, "wcat": wcat, "m1sr": m1sr, "ehc": ehc,
            "ones1": ones1_a, "ptc": ptb.astype(bf),
        })
        rhos.append(rho)

    run = _get_runner()
    results = run(in_maps)

    x_out = np.empty((B, N, C), np.float32)
    attn_map = np.empty((B, N, N), np.float32)
    for b in range(B):
        attn_map[b] = results[b]["attn"]
        x_out[b] = results[b]["z"][rhos[b]]
    return x_out, attn_map


# revision 32
# speedup vs baseline: 1.0004x; 1.0004x over previous
"""Trainium2 Bass kernel for nn_CAttention (sparse cluster attention).

Contract: kernel(**inputs) takes FULL unsharded numpy inputs and returns the
full output tuple (x_out [8,2048,512] f32, attn_map [8,2048,2048] f32).

Strategy: data-parallel over batch B=8 across the 8 NeuronCores. Per core:
  - attn_map = (x@Wq*SCALE) @ (x@Wk)^T computed dense (required output).
  - The attn @ v_s product collapses: within a cluster every row of the
    masked/normalized attn matrix is identical, so the [N,N]@[N,C] product
    reduces to 16 per-cluster weighted sums of v rows (rank-16), and the
    output projection factors through Pt = St^T @ Wproj (host-precomputed).
  - All permutations (argsort shuffle, the swapaxes/reshape scramble, the
    restore gather) are folded into host-side input prep / output gather.
TensorEngine streams run in bf16 (full rate, halves DMA/SBUF); accumulation
is fp32 in PSUM; softmax/normalization arithmetic is fp32.
"""

import numpy as np

import concourse.bass as bass
import concourse.tile as tile
from concourse import bacc, mybir

N = 2048
C = 512
NCL = 16
EPS = 1e-6
EPSN = EPS / N
SCALE = (C // 8) ** -0.5  # HEAD_DIM = 64

f32 = mybir.dt.float32
bf16 = mybir.dt.bfloat16

KC = C // 128   # 4 contraction chunks over channel dim
IC = N // 128   # 16 row chunks over token dim
JB = N // 512   # 4 free-dim blocks of 512 over tokens


def _build_nc(rep=1):
    nc = bacc.Bacc("TRN2", target_bir_lowering=False, debug=False, num_devices=8)

    # xcat[k] = [xst_k | xsst_k | xt_k] column-concatenated per 128-row chunk;
    # wcat[k] = [wq_k | wk_k | wv_k]; m1sr pre-interleaved [128, IC*NCL];
    # ptc = pt flattened [17, 4*C]; ehc = [epsc | hones] [128, 18]
    xcat = nc.dram_tensor("xcat", [KC, 128, 3 * N], bf16, kind="ExternalInput")
    wcat = nc.dram_tensor("wcat", [KC, 128, 3 * C], bf16, kind="ExternalInput")
    m1sr = nc.dram_tensor("m1sr", [128, IC * NCL], f32, kind="ExternalInput")
    ehc = nc.dram_tensor("ehc", [128, NCL + 2], bf16, kind="ExternalInput")
    ones1 = nc.dram_tensor("ones1", [1, C], bf16, kind="ExternalInput")
    ptc = nc.dram_tensor("ptc", [NCL + 1, 4 * C], bf16, kind="ExternalInput")
    attn = nc.dram_tensor("attn", [N, N], f32, kind="ExternalOutput")
    z = nc.dram_tensor("z", [N, C], f32, kind="ExternalOutput")

    with tile.TileContext(nc) as tc:
        for _r in range(rep):
            with (
                tc.tile_pool(name=f"pers{_r}", bufs=1) as pc,
                tc.tile_pool(name=f"work{_r}", bufs=3) as pw,
                tc.tile_pool(name=f"psum{_r}", bufs=1, space="PSUM") as pp,
            ):
                xc_t, wc_t = [], []
                qT_t, kT_t = [], []
                for k in range(KC):
                    xc_t.append(pc.tile([128, 3 * N], bf16, tag=f"xc{k}",
                                        name=f"xc{k}"))
                    wc_t.append(pc.tile([128, 3 * C], bf16, tag=f"wc{k}",
                                        name=f"wc{k}"))
                    qT_t.append(pc.tile([128, N], bf16, tag=f"qT{k}", name=f"qT{k}"))
                    kT_t.append(pc.tile([128, N], bf16, tag=f"kT{k}", name=f"kT{k}"))
                xst_t = [t[:, 0:N] for t in xc_t]
                xsst_t = [t[:, N:2 * N] for t in xc_t]
                xt_t = [t[:, 2 * N:3 * N] for t in xc_t]
                wv_t = [t[:, 0:C] for t in wc_t]
                wq_t = [t[:, C:2 * C] for t in wc_t]
                wk_t = [t[:, 2 * C:3 * C] for t in wc_t]

                # ---- loads: few large DMAs, first-consumption order ----
                for k in range(KC):
                    nc.scalar.dma_start(wc_t[k][:], wcat[k])
                for k in range(KC):
                    nc.sync.dma_start(xc_t[k][:, 0:512], xcat[k, :, 0:512])
                for k in range(KC):
                    nc.sync.dma_start(xc_t[k][:, N:N + 512], xcat[k, :, N:N + 512])
                for k in range(KC):
                    nc.sync.dma_start(xc_t[k][:, 2 * N:2 * N + 512],
                                      xcat[k, :, 2 * N:2 * N + 512])
                for k in range(KC):
                    nc.sync.dma_start(xc_t[k][:, 512:N], xcat[k, :, 512:N])
                for k in range(KC):
                    nc.sync.dma_start(xc_t[k][:, N + 512:2 * N],
                                      xcat[k, :, N + 512:2 * N])
                for k in range(KC):
                    nc.sync.dma_start(xc_t[k][:, 2 * N + 512:3 * N],
                                      xcat[k, :, 2 * N + 512:3 * N])
                m1s_sb = pc.tile([128, IC * NCL], f32, tag="m1s", name="m1s_sb")
                nc.scalar.dma_start(m1s_sb[:], m1sr[:])
                eh_sb = pc.tile([128, NCL + 2], bf16, tag="ehc", name="eh_sb")
                nc.scalar.dma_start(eh_sb[:], ehc[:])
                epsc_sb = eh_sb[:, 0:NCL]
                hones_sb = eh_sb[:, NCL:NCL + 2]
                ptc_sb = pc.tile([NCL + 1, 4 * C], bf16, tag="ptc", name="ptc_sb")
                nc.scalar.dma_start(ptc_sb[:], ptc[:])
                pt_sb = [ptc_sb[:, t4 * C:(t4 + 1) * C] for t4 in range(4)]
                o17 = pc.tile([NCL + 1, C], bf16, tag="o17", name="o17")
                nc.scalar.dma_start(o17[NCL:NCL + 1, :], ones1[:])

                ws_ps = pp.tile([NCL, C], f32, tag="ws", name="ws_ps")
                s_ps = pp.tile([NCL, 2], f32, tag="s", name="s_ps")

                # ---- phase 1 ic-loop with interleaved qT/kT projection ----
                proj_pairs = [(m, j) for m in range(KC) for j in range(JB)]
                pending = []
                for i in range(IC):
                    sl = bass.ts(i, 128)
                    ps_v = pp.tile([128, C], f32, tag="psv", name="ps_v")
                    ps_q = pp.tile([128, C], f32, tag="psq", name="ps_q")
                    ps_k = pp.tile([128, C], f32, tag="psk", name="ps_k")
                    for k in range(KC):
                        nc.tensor.matmul(ps_v[:], xst_t[k][:, sl], wv_t[k],
                                         start=(k == 0), stop=(k == KC - 1))
                    for k in range(KC):
                        nc.tensor.matmul(ps_q[:], xsst_t[k][:, sl], wq_t[k][:],
                                         start=(k == 0), stop=(k == KC - 1))
                    for k in range(KC):
                        nc.tensor.matmul(ps_k[:], xt_t[k][:, sl], wk_t[k][:],
                                         start=(k == 0), stop=(k == KC - 1))
                    # rowdot -> colv; exp -> e  (fp32 throughout)
                    # (tensor_tensor_reduce hangs on HW; use copy+mul+reduce)
                    qdd = pw.tile([128, C], f32, tag="qdd", name="qdd")
                    nc.vector.tensor_copy(qdd[:], ps_q[:])
                    nc.vector.tensor_mul(qdd[:], qdd[:], ps_k[:])
                    colv = pw.tile([128, 1], f32, tag="colv", name="colv")
                    nc.vector.reduce_sum(colv[:], qdd[:], axis=mybir.AxisListType.X)
                    e_sb = pw.tile([128, 1], f32, tag="e", name="e_sb")
                    nc.scalar.activation(e_sb[:], colv[:],
                                         mybir.ActivationFunctionType.Exp)
                    m1e = pw.tile([128, NCL], bf16, tag="m1e", bufs=3, name="m1e")
                    nc.vector.tensor_scalar_mul(
                        m1e[:], m1s_sb[:, bass.ts(i, NCL)], e_sb[:])
                    vs = pw.tile([128, C], bf16, tag="vs", bufs=3, name="vs")
                    nc.scalar.activation(vs[:], ps_v[:],
                                         mybir.ActivationFunctionType.Copy)
                    pending.append((m1e, vs))
                    if i > 0:
                        pm1e, pvs = pending.pop(0)
                        first = (i == 1)
                        nc.tensor.matmul(ws_ps[:], pm1e[:], pvs[:],
                                         start=first, stop=False,
                                         skip_group_check=True)
                        nc.tensor.matmul(ws_ps[:], epsc_sb[:], pvs[:],
                                         start=False, stop=False,
                                         skip_group_check=True)
                        nc.tensor.matmul(s_ps[:], pm1e[:], hones_sb[:],
                                         start=first, stop=False,
                                         skip_group_check=True)
                    # interleave one qT/kT projection (m, j) pair per ic
                    m, j = proj_pairs[i]
                    slj = bass.ts(j, 512)
                    psp = pp.tile([128, 512], f32, tag="mm", bufs=3, name="psp")
                    for k in range(KC):
                        nc.tensor.matmul(psp[:], wq_t[k][:, bass.ts(m, 128)],
                                         xt_t[k][:, slj],
                                         start=(k == 0), stop=(k == KC - 1))
                    nc.scalar.activation(qT_t[m][:, slj], psp[:],
                                         mybir.ActivationFunctionType.Copy)
                    psp2 = pp.tile([128, 512], f32, tag="mm", bufs=3, name="psp2")
                    for k in range(KC):
                        nc.tensor.matmul(psp2[:], wk_t[k][:, bass.ts(m, 128)],
                                         xt_t[k][:, slj],
                                         start=(k == 0), stop=(k == KC - 1))
                    nc.vector.tensor_copy(kT_t[m][:, slj], psp2[:])

                pm1e, pvs = pending.pop(0)
                nc.tensor.matmul(ws_ps[:], pm1e[:], pvs[:],
                                 start=False, stop=False,
                                 skip_group_check=True)
                nc.tensor.matmul(ws_ps[:], epsc_sb[:], pvs[:],
                                 start=False, stop=True,
                                 skip_group_check=True)
                nc.tensor.matmul(s_ps[:], pm1e[:], hones_sb[:],
                                 start=False, stop=True,
                                 skip_group_check=True)

                # ---- O = (WS + eps-term) / (S + EPS); bias row is 1.0 ----
                s_eps = pw.tile([NCL, 1], f32, tag="seps", name="s_eps")
                nc.vector.tensor_scalar_add(s_eps[:], s_ps[:, 0:1], EPS)
                recip = pw.tile([NCL, 1], f32, tag="recip", name="recip")
                nc.vector.reciprocal(recip[:], s_eps[:])
                nc.vector.tensor_scalar_mul(o17[0:NCL, :], ws_ps[:], recip[:])

                # ---- attn_map = qT^T @ kT, streamed out (merged DMA) ----
                # z units (Z_t = O'^T @ Pt, rank-17) interleaved into the loop
                zv = z[:].rearrange("(p t) m -> t p m", t=4)
                zunits = [(t4, pck) for t4 in range(4) for pck in range(KC)]
                for i in range(IC):
                    sli = bass.ts(i, 128)
                    asb = pw.tile([128, N], f32, tag="asb", bufs=4, name="asb")
                    for j in range(JB):
                        slj = bass.ts(j, 512)
                        psa = pp.tile([128, 512], f32, tag="mm", bufs=3,
                                      name="psa")
                        for m in range(KC):
                            nc.tensor.matmul(psa[:], qT_t[m][:, sli],
                                             kT_t[m][:, slj],
                                             start=(m == 0), stop=(m == KC - 1))
                        if j % 2 == 0:
                            nc.vector.tensor_copy(asb[:, slj], psa[:])
                        else:
                            nc.scalar.activation(
                                asb[:, slj], psa[:],
                                mybir.ActivationFunctionType.Copy)
                    if i >= IC - 2:
                        for jj in range(JB):
                            sjj = bass.ts(jj, 512)
                            eng = nc.sync if jj % 2 == 0 else nc.scalar
                            eng.dma_start(attn[sli, sjj], asb[:, sjj])
                    elif i % 2 == 0:
                        nc.sync.dma_start(attn[sli, :], asb[:])
                    else:
                        nc.scalar.dma_start(attn[sli, :], asb[:])
                    if i < 8:
                        for t4, pck in (zunits[2 * i], zunits[2 * i + 1]):
                            psz = pp.tile([128, C], f32, tag="mm", bufs=3,
                                          name="psz")
                            nc.tensor.matmul(psz[:], o17[:, bass.ts(pck, 128)],
                                             pt_sb[t4], start=True, stop=True)
                            zsb = pw.tile([128, C], f32, tag="zsb", bufs=2,
                                          name="zsb")
                            if pck % 2 == 0:
                                nc.vector.tensor_copy(zsb[:], psz[:])
                            else:
                                nc.scalar.activation(
                                    zsb[:], psz[:],
                                    mybir.ActivationFunctionType.Copy)
                            nc.sync.dma_start(zv[t4, bass.ts(pck, 128), :],
                                              zsb[:])

    nc.compile()
    return nc


def _make_runner(nc, n_cores=8):
    import jax
    from jax.sharding import Mesh, PartitionSpec
    from jax.experimental.shard_map import shard_map
    from concourse import bass2jax
    from concourse.bass2jax import _bass_exec_p, install_neuronx_cc_hook

    install_neuronx_cc_hook()
    partition_name = nc.partition_id_tensor.name if nc.partition_id_tensor else None
    in_names, out_names, out_avals, zero_outs = [], [], [], []
    for alloc in nc.m.functions[0].allocations:
        if not isinstance(alloc, mybir.MemoryLocationSet):
            continue
        name = alloc.memorylocations[0].name
        if alloc.kind == "ExternalInput":
            if name != partition_name:
                in_names.append(name)
        elif alloc.kind == "ExternalOutput":
            out_names.append(name)
            shape = tuple(alloc.tensor_shape)
            dtype = mybir.dt.np(alloc.dtype)
            out_avals.append(jax.core.ShapedArray(shape, dtype))
            zero_outs.append(np.zeros(shape, dtype))
    n_params = len(in_names)
    n_outs = len(out_avals)
    all_in_names = list(in_names) + list(out_names)
    if partition_name is not None:
        all_in_names.append(partition_name)

    def _body(*args):
        operands = list(args)
        if partition_name is not None:
            operands.append(bass2jax.partition_id_tensor())
        outs = _bass_exec_p.bind(
            *operands,
            out_avals=tuple(out_avals),
            in_names=tuple(all_in_names),
            out_names=tuple(out_names),
            lowering_input_output_aliases=(),
            sim_require_finite=True,
            sim_require_nnan=True,
            nc=nc,
        )
        return tuple(outs)

    devices = jax.devices()[:n_cores]
    mesh = Mesh(np.asarray(devices), ("core",))
    in_specs = (PartitionSpec("core"),) * (n_params + n_outs)
    out_specs = (PartitionSpec("core"),) * n_outs
    sharded = jax.jit(
        shard_map(_body, mesh=mesh, in_specs=in_specs, out_specs=out_specs,
                  check_rep=False),
        keep_unused=True,
    )

    def run(in_maps):
        per_core = [[np.asarray(m[name]) for name in in_names] for m in in_maps]
        concat_in = [
            np.concatenate([per_core[cc][i] for cc in range(n_cores)], axis=0)
            for i in range(n_params)
        ]
        concat_zeros = [
            np.zeros((n_cores * zz.shape[0], *zz.shape[1:]), zz.dtype)
            for zz in zero_outs
        ]
        out_arrs = sharded(*concat_in, *concat_zeros)
        import jax as _jax
        _jax.block_until_ready(out_arrs)
        return [
            {name: np.asarray(out_arrs[i]).reshape(n_cores, *out_avals[i].shape)[cc]
             for i, name in enumerate(out_names)}
            for cc in range(n_cores)
        ]

    return run


_STATE = {}


def _get_runner():
    if "run" not in _STATE:
        nc = _build_nc()
        _STATE["nc"] = nc
        _STATE["run"] = _make_runner(nc, 8)
    return _STATE["run"]


def kernel(x_token, x_path, idx_cluster, cluster_num, Wqk, Wv, Wpv, Wproj, bproj):
    import ml_dtypes
    bf = ml_dtypes.bfloat16
    x = np.asarray(x_token, dtype=np.float32)
    idx = np.asarray(idx_cluster)
    B = x.shape[0]
    cn = int(cluster_num)
    Wq = np.ascontiguousarray(np.asarray(Wqk, np.float32)[:, :C] * SCALE)
    Wk = np.ascontiguousarray(np.asarray(Wqk, np.float32)[:, C:])
    Wv_ = np.asarray(Wv, np.float32)
    Wp = np.asarray(Wproj, np.float32)
    bp = np.asarray(bproj, np.float32)
    # wcat[k] = [wq_k | wk_k | wv_k]  (bf16)
    wcat = np.empty((KC, 128, 3 * C), np.float32)
    for k in range(KC):
        r = slice(k * 128, (k + 1) * 128)
        wcat[k, :, 0:C] = Wv_[r]
        wcat[k, :, C:2 * C] = Wq[r]
        wcat[k, :, 2 * C:3 * C] = Wk[r]
    wcat = wcat.astype(bf)
    ehc = np.concatenate([np.full((128, NCL), EPSN, np.float32),
                          np.ones((128, 2), np.float32)], axis=1).astype(bf)
    ones1_a = np.ones((1, C), bf)

    in_maps = []
    rhos = []
    for b in range(B):
        sig = np.argsort(idx[b], kind="stable")
        rho = np.argsort(sig, kind="stable")
        s = idx[b][sig]
        xb = x[b]
        xT = xb.T
        xcat = np.empty((KC, 128, 3 * N), np.float32)
        xsT = xb[sig].T
        xssT = xb[sig[sig]].T
        for k in range(KC):
            r = slice(k * 128, (k + 1) * 128)
            xcat[k, :, 0:N] = xsT[r]
            xcat[k, :, N:2 * N] = xssT[r]
            xcat[k, :, 2 * N:3 * N] = xT[r]
        xcat = xcat.astype(bf)
        m1 = np.zeros((N, NCL), np.float32)
        act = s < cn
        m1[np.nonzero(act)[0], s[act]] = 1.0
        # pre-interleave: m1sr[p, i*NCL + c] = m1[i*128 + p, c]
        m1sr = np.ascontiguousarray(
            m1.reshape(IC, 128, NCL).transpose(1, 0, 2).reshape(128, IC * NCL))
        ptb = np.zeros((NCL + 1, 4 * C), np.float32)
        for t4 in range(4):
            st = s[t4 * 512:(t4 + 1) * 512]
            blk = np.zeros((NCL + 1, C), np.float32)
            np.add.at(blk, st, Wp)
            blk[NCL] = bp
            ptb[:, t4 * C:(t4 + 1) * C] = blk
        in_maps.append({
            "xcat": xcat, "wcat": wcat, "m1sr": m1sr, "ehc": ehc,
            "ones1": ones1_a, "ptc": ptb.astype(bf),
        })
        rhos.append(rho)

    run = _get_runner()
    results = run(in_maps)

    x_out = np.empty((B, N, C), np.float32)
    attn_map = np.empty((B, N, N), np.float32)
    for b in range(B):
        attn_map[b] = results[b]["attn"]
        x_out[b] = results[b]["z"][rhos[b]]
    return x_out, attn_map
